# revision 17
# baseline (speedup 1.0000x reference)
"""Multi-head self-attention on 8 Trainium2 NeuronCores.

Strategy (batch x head-group sharding):
  - 2 batches x 4 head-groups -> each core owns batch b = core//4 and
    heads 4g..4g+3 (g = core%4): a 256-column slice of Wq/Wk/Wv and the
    matching 256-row slice of Wo, applied to one batch's tokens.
  - All matmul operands are bf16 (cast on the host): full PE rate and
    half the SBUF/DMA traffic of fp32.  (fp8 was numerically tested and
    rejected: e4m3 scores -> 3.1e-2 rel err, over the 2e-2 gate.)
  - Q/K/V are projected in [e, s] layout (weights stationary); V is
    then flipped to [s, e] via the DMA xbar transpose engine, then
    strided DVE copies into the ones-augmented AV layout [V_h | 1].
  - Scores are computed transposed, ST[k, q] = K^T Q, two heads
    row-packed into the PE array (64-wide contraction per head).
  - softmax exp: most k-tiles on ACT (table exp, bf16 out); selected
    k-tiles in ACT-paced blocks go to the otherwise-idle GPSIMD/Pool
    engine via the Schraudolph bit trick (affine fp32->int32
    tensor_scalar, then an f32r rounding copy).  The softmax
    denominator rides the AV matmul via the ones column.
  - Per-head normalization happens on the transposed attention matrix
    right before the output projection; partial outputs are written
    bf16 and the 4 per-batch partials are summed on host (the Wo
    row-parallel all-reduce) with bo added there.

Schedule (engines execute in emission order, so placement == schedule):
  Input DMA is split into per-o-chunk pieces, ordered by first use and
  spread across three queues (sync HWDGE, scalar HWDGE, gpsimd SWDGE)
  so the lead-in K/Q projections start ~10us in, overlapping the
  HBM-bound input load (~6.5MB @ ~350GB/s).  The lead-in projects K
  sl0 / Q sl0 for the first 1024 tokens, then attention pair 0 streams
  while remaining projection tiles are emitted (in half-tile items)
  into kt slots of the blocks.  AV matmuls trail the exp stream by
  `lag` k-tiles.  Consecutive blocks are software-pipelined: block N+1's
  kt0 scores+exp are emitted before block N's tail-AV drain, so the
  in-order PE never sits behind the drain waiting on exp.
  Pair 1 blocks carry the output projections of earlier q-chunks; the
  last block defers half its carried outproj into the drain region to
  cover the final normalize.  Output DMAs alternate sync/scalar queues.
PSUM: scores 2x[128,1024] (4 banks) + AV accumulators 2x[65,512]
  (2 banks) + single-buffered proj/outproj [128,1024] (2 banks) = 8.
"""
import sys

sys.path.insert(0, "/opt/trn_rl_repo")

import numpy as np
import ml_dtypes

import concourse.bacc as bacc
import concourse.tile as tile
from concourse import mybir
from concourse.bass_utils import run_bass_kernel_spmd

AF = mybir.ActivationFunctionType
F32 = mybir.dt.float32
F32R = mybir.dt.float32r
I32 = mybir.dt.int32
BF = mybir.dt.bfloat16
BF_NP = ml_dtypes.bfloat16

N_CORES = 8
D = 1024          # model dim
S = 2048          # tokens per core (one batch)
E = 256           # per-core projection width (4 heads x 64)
HD = 64           # head dim
P = 128           # partitions
QC = 512          # q-chunk
SC = 1024         # projection s-chunk
DC = D // P       # 8
N_KT = S // P     # 16
N_QC = S // QC    # 4
N_SC = S // SC    # 2
EW = HD + 1       # per-head V width with ones column

SCHR_A = float(2**23 / np.log(2.0)) / 8.0          # folds the 1/8 scale
SCHR_B = float(127 * 2**23 - 0.043677448 * 2**23 + 0.5)


def build_attention_core(with_qkv_bias=False):
    scale = 1.0 / np.sqrt(np.float32(HD))

    nc = bacc.Bacc("TRN2", target_bir_lowering=False)
    xT = nc.dram_tensor("xT", [P, DC, S], BF, kind="ExternalInput")
    wq = nc.dram_tensor("wq", [P, DC, E], BF, kind="ExternalInput")
    wk = nc.dram_tensor("wk", [P, DC, E], BF, kind="ExternalInput")
    wv = nc.dram_tensor("wv", [P, DC, E], BF, kind="ExternalInput")
    wo = nc.dram_tensor("wo", [P, 2, D], BF, kind="ExternalInput")
    bq = nc.dram_tensor("bq", [P, 2], F32, kind="ExternalInput")
    bk = nc.dram_tensor("bk", [P, 2], F32, kind="ExternalInput")
    bv = nc.dram_tensor("bv", [P, 2], F32, kind="ExternalInput")
    out = nc.dram_tensor("out", [S, D], BF, kind="ExternalOutput")

    with tile.TileContext(nc) as tc:
        with (
            tc.tile_pool(name="persist", bufs=1) as persist,
            tc.tile_pool(name="attp", bufs=6) as attp,
            tc.tile_pool(name="upool", bufs=8) as upool,
            tc.tile_pool(name="u32p", bufs=1) as u32p,
            tc.tile_pool(name="urp", bufs=2) as urp,
            tc.tile_pool(name="vtrp", bufs=2) as vtrp,
            tc.tile_pool(name="small", bufs=2) as small,
            tc.tile_pool(name="outp", bufs=2) as outp,
            tc.tile_pool(name="psS", bufs=2, space="PSUM") as psS,
            tc.tile_pool(name="psP", bufs=2, space="PSUM") as psP,
            tc.tile_pool(name="psQ", bufs=1, space="PSUM") as psQ,
        ):
            # ---- input DMAs: consumption-ordered chunks on 3 queues ------
            w_sb = {}
            for nm in ("k", "v", "q"):
                w_sb[nm] = persist.tile([P, DC, E], BF, tag=f"w_{nm}",
                                        name=f"w_{nm}")
            wo_sb = persist.tile([P, 2, D], BF)
            x_sb = persist.tile([P, DC, S], BF)

            # Medium chunks (DMA issue costs ~0.7us of queue time each), in
            # first-use order, spread over sync/scalar HWDGE + gpsimd SWDGE.
            # sync: wk first half, x(o0-1 sc0), x(o4-5 sc0), wq first half,
            #       then late x(sc1).
            nc.sync.dma_start(w_sb["k"][:, 0:4], wk[:, 0:4])
            nc.sync.dma_start(x_sb[:, 0:2, 0:SC], xT[:, 0:2, 0:SC])
            nc.sync.dma_start(x_sb[:, 4:6, 0:SC], xT[:, 4:6, 0:SC])
            nc.sync.dma_start(w_sb["q"][:, 0:4], wq[:, 0:4])
            nc.sync.dma_start(x_sb[:, 6:8, SC:2 * SC], xT[:, 6:8, SC:2 * SC])
            # scalar: wk second half, x(o2-3 sc0), x(o6-7 sc0), wq second
            #         half, early x(sc1), wo.
            nc.scalar.dma_start(w_sb["k"][:, 4:8], wk[:, 4:8])
            nc.scalar.dma_start(x_sb[:, 2:4, 0:SC], xT[:, 2:4, 0:SC])
            nc.scalar.dma_start(x_sb[:, 6:8, 0:SC], xT[:, 6:8, 0:SC])
            nc.scalar.dma_start(w_sb["q"][:, 4:8], wq[:, 4:8])
            nc.scalar.dma_start(x_sb[:, 0:2, SC:2 * SC], xT[:, 0:2, SC:2 * SC])
            nc.scalar.dma_start(wo_sb[:], wo[:])
            # gpsimd SWDGE: wv whole, mid x(sc1).
            nc.gpsimd.dma_start(w_sb["v"][:], wv[:])
            nc.gpsimd.dma_start(x_sb[:, 2:4, SC:2 * SC], xT[:, 2:4, SC:2 * SC])
            nc.gpsimd.dma_start(x_sb[:, 4:6, SC:2 * SC], xT[:, 4:6, SC:2 * SC])

            bias_t = {}
            if with_qkv_bias:
                for nm, t in (("q", bq), ("k", bk), ("v", bv)):
                    bt = persist.tile([P, 2], F32, tag=f"b_{nm}")
                    nc.gpsimd.dma_start(bt[:], t[:])
                    bias_t[nm] = bt

            # ---- persistent activations ----------------------------------
            KT = persist.tile([P, 2, S], BF, tag="KT")   # [e, slice, s]
            QT = persist.tile([P, 2, S], BF, tag="QT")
            VT = persist.tile([P, 2, S], BF, tag="VT")
            # AV stationary: per k-chunk [V_h0|1|V_h1|1|V_h2|1|V_h3|1]
            V_sb = persist.tile([P, N_KT, 4 * EW], BF, tag="V")
            V_r = V_sb[:].rearrange("p c (h u) -> p c h u", u=EW)
            V32 = persist.tile([P, N_KT, 4 * EW], F32R, tag="V32")
            V32_r = V32[:].rearrange("p c (h u) -> p c h u", u=EW)
            V32f_r = V32[:].bitcast(F32).rearrange("p c (h u) -> p c h u", u=EW)
            for h in range(4):
                nc.gpsimd.memset(V_r[:, :, h, HD], 1.0)
                nc.gpsimd.memset(V32f_r[:, :, h, HD], 1.0)

            # ---- projection emitters (two-half items) --------------------
            dsts = {"k": KT, "q": QT, "v": VT}

            def proj_h1(nm, sl, sc, pool=None):
                s0 = sc * SC
                ps = (pool or psQ).tile([P, SC], F32,
                                        tag="S" if pool else "Q",
                                        name=f"ps_{nm}")
                for o in range(DC // 2):
                    for hh in range(SC // 512):
                        nc.tensor.matmul(
                            ps[:, hh * 512:(hh + 1) * 512],
                            w_sb[nm][:, o, sl * P:(sl + 1) * P],
                            x_sb[:, o, s0 + hh * 512:s0 + (hh + 1) * 512],
                            start=(o == 0), stop=False,
                        )
                return ps

            def proj_h2(nm, sl, sc, ps):
                s0 = sc * SC
                for o in range(DC // 2, DC):
                    for hh in range(SC // 512):
                        nc.tensor.matmul(
                            ps[:, hh * 512:(hh + 1) * 512],
                            w_sb[nm][:, o, sl * P:(sl + 1) * P],
                            x_sb[:, o, s0 + hh * 512:s0 + (hh + 1) * 512],
                            start=False, stop=(o == DC - 1),
                        )
                dst = dsts[nm][:, sl, s0:s0 + SC]
                if with_qkv_bias:
                    nc.vector.tensor_tensor(
                        dst, ps[:],
                        bias_t[nm][:, sl:sl + 1].to_broadcast((P, SC)),
                        mybir.AluOpType.add)
                else:
                    nc.vector.tensor_copy(dst, ps[:])

            def emit_proj(nm, sl, sc):
                proj_h2(nm, sl, sc, proj_h1(nm, sl, sc))

            def emit_vflip(h, sc):
                """Transpose head h's V tokens [sc*SC,(sc+1)*SC) into V_sb."""
                sl, h2 = divmod(h, 2)
                vtr = vtrp.tile([P, SC // P, HD], BF, tag="vtr")
                nc.sync.dma_start_transpose(
                    vtr[:],
                    VT[h2 * HD:(h2 + 1) * HD, sl, sc * SC:(sc + 1) * SC])
                c0 = sc * (SC // P)
                nc.vector.tensor_copy(
                    V_r[:, c0:c0 + SC // P, h, 0:HD], vtr[:])
                nc.vector.tensor_copy(
                    V32_r[:, c0:c0 + SC // P, h, 0:HD], vtr[:])

            # ---- attention -----------------------------------------------
            def emit_scores_exp(p, qc, kt, schr):
                """Scores + exp for one k-tile; returns a pend entry."""
                q0 = qc * QC
                k0 = kt * P
                st = psS.tile([P, 2 * QC], F32, tag="S", name="st")
                nc.tensor.matmul(
                    st[:, 0:QC],
                    KT[0:HD, p, k0:k0 + P], QT[0:HD, p, q0:q0 + QC],
                    tile_position=(0, 0), start=True, stop=True)
                nc.tensor.matmul(
                    st[:, QC:2 * QC],
                    KT[HD:P, p, k0:k0 + P], QT[HD:P, p, q0:q0 + QC],
                    tile_position=(64, 0), start=True, stop=True)
                if schr:
                    # Affine fp32->int32 on DVE (Pool can't read PSUM); the
                    # f32r rounding copy on the otherwise-idle GPSIMD (the
                    # BIR verifier requires a true f32r-rounding producer).
                    u32 = u32p.tile([P, 2 * QC], I32, tag="U32")
                    nc.vector.tensor_scalar(
                        u32[:], st[:], SCHR_A, SCHR_B,
                        mybir.AluOpType.mult, mybir.AluOpType.add)
                    ur = urp.tile([P, 2 * QC], F32R, tag="UR")
                    nc.gpsimd.tensor_copy(ur[:], u32[:].bitcast(F32))
                    return (kt, ur, True)
                ut = upool.tile([P, 2 * QC], BF, tag="U")
                nc.scalar.activation(ut[:], st[:], AF.Exp,
                                     scale=float(scale))
                return (kt, ut, False)

            def emit_block(p, qc, sched, lag, catchup, schr_kts,
                           head=(), next_head_fn=None):
                """Scores+exp+AV for head pair p, q-chunk qc.

                sched: {kt: [callables]} -- projection/outproj work emitted
                into that kt slot ('post' runs after the AV drain).  AV
                trails exp by `lag` k-tiles (catching up from kt=catchup).
                head: pend entries pre-emitted by the previous block.
                next_head_fn: emits the next block's kt0 just before this
                block's tail-AV drain (software pipelining).
                """
                pa = [psP.tile([EW, QC], F32, tag="P", name=f"pa{h}")
                      for h in range(2)]

                def emit_av(kt, ut, f32r):
                    vsrc = V32 if f32r else V_sb
                    for h in range(2):
                        nc.tensor.matmul(
                            pa[h][:],
                            vsrc[:, kt, (2 * p + h) * EW:(2 * p + h + 1) * EW],
                            ut[:, h * QC:(h + 1) * QC],
                            start=(kt == 0), stop=(kt == N_KT - 1))

                pend = list(head)
                for kt in range(N_KT):
                    if kt >= len(head):
                        pend.append(emit_scores_exp(p, qc, kt, kt in schr_kts))
                    hi = kt - lag
                    if catchup is not None:
                        hi += max(0, kt - catchup)
                    while pend and pend[0][0] <= hi:
                        emit_av(*pend.pop(0))
                    for fn in sched.get(kt, ()):
                        fn()
                nh = next_head_fn() if next_head_fn else ()
                for item in pend:
                    emit_av(*item)
                for fn in sched.get('post', ()):
                    fn()
                return pa, nh

            def emit_tail(p, qc, pa):
                """Normalize pair p's attention -> attnT (bf16, persists)."""
                rsb = small.tile([1, 2 * QC], F32, tag="rsb")
                for h in range(2):
                    nc.vector.tensor_copy(
                        rsb[0:1, h * QC:(h + 1) * QC], pa[h][HD:EW, :])
                rinv1 = small.tile([1, 2 * QC], F32, tag="rinv1")
                nc.vector.reciprocal_approx_fast(rinv1[:], rsb[:])
                rb = small.tile([HD, 2 * QC], F32, tag="rb")
                nc.gpsimd.partition_broadcast(rb[:], rinv1[0:1, :])
                attnT = attp.tile([P, QC], BF, tag=f"attnT_{p}_{qc}")
                for h in range(2):
                    nc.vector.tensor_tensor(
                        attnT[h * HD:(h + 1) * HD, :],
                        pa[h][0:HD, :], rb[:, h * QC:(h + 1) * QC],
                        mybir.AluOpType.mult)
                return attnT

            _out_q = [0]

            def out_dma(dst, src):
                # Keep output stores OFF the ACT queue (exp stream lives
                # there); alternate the idle sync HWDGE and gpsimd SWDGE.
                eng = nc.sync if _out_q[0] % 2 == 0 else nc.gpsimd
                _out_q[0] += 1
                eng.dma_start(dst, src)

            def emit_outproj_ss(qc, ss, attnT_by_p):
                q0 = qc * QC
                po = psQ.tile([P, D], F32, tag="Q", name="po")
                for p in range(2):
                    for oc in range(D // 512):
                        nc.tensor.matmul(
                            po[:, oc * 512:(oc + 1) * 512],
                            attnT_by_p[p][:, ss * P:(ss + 1) * P],
                            wo_sb[:, p, oc * 512:(oc + 1) * 512],
                            start=(p == 0), stop=(p == 1))
                osb = outp.tile([P, D], BF, tag="osb", name="osb")
                nc.vector.tensor_copy(osb[:], po[:])
                out_dma(out[q0 + ss * P:q0 + (ss + 1) * P, :], osb[:])

            def emit_outproj_final(qc, attnT_by_p):
                # Tail outproj: [128,512] units pipelined 2-deep through the
                # pa slots (free once the tail normalize has consumed them);
                # full-width bf16 rows assembled so the out DMA is contiguous.
                q0 = qc * QC
                for ss in range(QC // P):
                    osb = outp.tile([P, D], BF, tag="osb", name="osb_f")
                    for oc in range(D // 512):
                        po = psP.tile([P, 512], F32, tag="P", name="po2")
                        for p in range(2):
                            nc.tensor.matmul(
                                po[:],
                                attnT_by_p[p][:, ss * P:(ss + 1) * P],
                                wo_sb[:, p, oc * 512:(oc + 1) * 512],
                                start=(p == 0), stop=(p == 1))
                        nc.vector.tensor_copy(
                            osb[:, oc * 512:(oc + 1) * 512], po[:])
                    out_dma(out[q0 + ss * P:q0 + (ss + 1) * P, :], osb[:])

            # ---- schedule ------------------------------------------------
            # Lead-in: K00 accumulates in a psS buffer (scores haven't
            # started, the pool is idle) so Q00 in psQ isn't serialized
            # behind K00's PSUM->SBUF copy by the single psQ buffer.
            proj_h2("k", 0, 0, proj_h1("k", 0, 0, pool=psS))
            emit_proj("q", 0, 0)

            ctx = {}

            def H1(nm, sl, sc):
                return lambda: ctx.__setitem__(
                    (nm, sl, sc), proj_h1(nm, sl, sc))

            def H2(nm, sl, sc):
                return lambda: proj_h2(nm, sl, sc, ctx.pop((nm, sl, sc)))

            F = lambda h, sc: (lambda: emit_vflip(h, sc))

            attnT = {}

            def OP(qc, ss):
                return lambda: emit_outproj_ss(
                    qc, ss, (attnT[(0, qc)], attnT[(1, qc)]))

            # blocks: (p, qc, sched, lag, catchup, schr_kts)
            blocks = [
                # p0 qc0: V00 early, K01/V01 after x(sc1) lands, flips late.
                (0, 0, {2: [H1("v", 0, 0)], 3: [H2("v", 0, 0)],
                        4: [H1("k", 0, 1)], 5: [F(0, 0)],
                        6: [F(1, 0)], 7: [H2("k", 0, 1)],
                        8: [H1("v", 0, 1)], 9: [H2("v", 0, 1)],
                        11: [F(0, 1)], 12: [F(1, 1)]},
                 8, None, ()),
                (0, 1, {0: [H1("q", 0, 1)], 2: [H2("q", 0, 1)],
                        4: [H1("k", 1, 0)], 6: [H2("k", 1, 0)],
                        8: [H1("v", 1, 0)], 10: [H2("v", 1, 0)],
                        12: [F(2, 0)], 13: [F(3, 0)]},
                 5, 8, ()),
                (0, 2, {}, 5, 8, (2, 7, 12)),
                (0, 3, {0: [H1("q", 1, 0)], 2: [H2("q", 1, 0)]},
                 5, 8, (7,)),
                (1, 0, {0: [H1("k", 1, 1)], 2: [H2("k", 1, 1)],
                        4: [H1("v", 1, 1)], 6: [H2("v", 1, 1)],
                        8: [F(2, 1)], 10: [F(3, 1)]},
                 5, 8, ()),
                (1, 1, {0: [H1("q", 1, 1)], 2: [H2("q", 1, 1)],
                        4: [OP(0, 0)], 7: [OP(0, 1)],
                        10: [OP(0, 2)], 13: [OP(0, 3)]},
                 5, 8, ()),
                (1, 2, {4: [OP(1, 0)], 7: [OP(1, 1)],
                        10: [OP(1, 2)], 13: [OP(1, 3)]},
                 5, 8, (2, 9)),
                (1, 3, {4: [OP(2, 0)], 7: [OP(2, 1)],
                        10: [OP(2, 2)], 13: [OP(2, 3)]},
                 5, 8, (2, 9)),
            ]

            head = ()
            for i, (p, qc, sched, lag, cu, schr) in enumerate(blocks):
                if i + 1 < len(blocks):
                    np_, nqc, _, _, _, nschr = (
                        blocks[i + 1][0], blocks[i + 1][1], None, None, None,
                        blocks[i + 1][5])

                    def next_head_fn(np_=np_, nqc=nqc, nschr=nschr):
                        return [emit_scores_exp(np_, nqc, 0, 0 in nschr)]
                else:
                    next_head_fn = None
                pa, head = emit_block(p, qc, sched, lag, cu, schr,
                                      head=head, next_head_fn=next_head_fn)
                attnT[(p, qc)] = emit_tail(p, qc, pa)

            emit_outproj_final(3, (attnT[(0, 3)], attnT[(1, 3)]))

    nc.compile()
    return nc


_NC_CACHE = {}


def _get_nc(with_qkv_bias):
    key = with_qkv_bias
    if key not in _NC_CACHE:
        _NC_CACHE[key] = build_attention_core(with_qkv_bias)
    return _NC_CACHE[key]


def _pack_pdm(a):
    """[D, M] -> [128, D//128, M] partition-major, bf16."""
    Dd, M = a.shape
    return np.ascontiguousarray(
        a.reshape(Dd // P, P, M).transpose(1, 0, 2).astype(BF_NP))


def run_attention(x, Wq, bq, Wk, bk, Wv, bv, Wo, bo, trace=False):
    B, S_, D_ = x.shape
    assert (B, S_, D_) == (2, S, D)
    with_qkv_bias = bool(np.any(bq) or np.any(bk) or np.any(bv))
    nc = _get_nc(with_qkv_bias)
    in_maps = []
    for c in range(N_CORES):
        b, g = divmod(c, N_CORES // 2)
        sl = slice(g * E, (g + 1) * E)
        xTb = np.ascontiguousarray(x[b].T)  # [D, S]
        in_maps.append({
            "xT": _pack_pdm(xTb),
            "wq": _pack_pdm(Wq[:, sl]),
            "wk": _pack_pdm(Wk[:, sl]),
            "wv": _pack_pdm(Wv[:, sl]),
            "wo": np.ascontiguousarray(
                Wo[sl, :].reshape(2, P, D).transpose(1, 0, 2)
                .astype(BF_NP)),
            "bq": np.ascontiguousarray(
                bq[sl].reshape(2, P).T.astype(np.float32)),
            "bk": np.ascontiguousarray(
                bk[sl].reshape(2, P).T.astype(np.float32)),
            "bv": np.ascontiguousarray(
                bv[sl].reshape(2, P).T.astype(np.float32)),
        })
    res = run_bass_kernel_spmd(nc, in_maps, core_ids=list(range(N_CORES)),
                               trace=trace)
    outs = []
    for b in range(2):
        acc = np.zeros((S, D), dtype=np.float32)
        for g in range(N_CORES // 2):
            acc += np.asarray(res.results[b * 4 + g]["out"]).astype(np.float32)
        outs.append(acc + np.asarray(bo, dtype=np.float32)[None, :])
    return np.stack(outs).reshape(B, S, D), res


def kernel(x, Wq, bq, Wk, bk, Wv, bv, Wo, bo):
    out, _ = run_attention(np.asarray(x), np.asarray(Wq), np.asarray(bq),
                           np.asarray(Wk), np.asarray(bk), np.asarray(Wv),
                           np.asarray(bv), np.asarray(Wo), np.asarray(bo))
    return out


# revision 19
# speedup vs baseline: 1.0092x; 1.0092x over previous
"""Multi-head self-attention on 8 Trainium2 NeuronCores.

Strategy (batch x head-group sharding):
  - 2 batches x 4 head-groups -> each core owns batch b = core//4 and
    heads 4g..4g+3 (g = core%4): a 256-column slice of Wq/Wk/Wv and the
    matching 256-row slice of Wo, applied to one batch's tokens.
  - All matmul operands are bf16 (cast on the host): full PE rate and
    half the SBUF/DMA traffic of fp32.  (fp8 was numerically tested and
    rejected: e4m3 scores -> 3.1e-2 rel err, over the 2e-2 gate.)
  - Q/K/V are projected in [e, s] layout (weights stationary); V is
    then flipped to [s, e] via the DMA xbar transpose engine, then
    strided DVE copies into the ones-augmented AV layout [V_h | 1].
  - Scores are computed transposed, ST[k, q] = K^T Q, two heads
    row-packed into the PE array (64-wide contraction per head).
  - softmax exp: most k-tiles on ACT (table exp, bf16 out); selected
    k-tiles in ACT-paced blocks go to the otherwise-idle GPSIMD/Pool
    engine via the Schraudolph bit trick (affine fp32->int32
    tensor_scalar, then an f32r rounding copy).  The softmax
    denominator rides the AV matmul via the ones column.
  - Per-head normalization happens on the transposed attention matrix
    right before the output projection; partial outputs are written
    bf16 and the 4 per-batch partials are summed on host (the Wo
    row-parallel all-reduce) with bo added there.

Schedule (engines execute in emission order, so placement == schedule):
  Input DMA is split into per-o-chunk pieces, ordered by first use and
  spread across three queues (sync HWDGE, scalar HWDGE, gpsimd SWDGE)
  so the lead-in K/Q projections start ~10us in, overlapping the
  HBM-bound input load (~6.5MB @ ~350GB/s).  The lead-in projects K
  sl0 / Q sl0 for the first 1024 tokens, then attention pair 0 streams
  while remaining projection tiles are emitted (in half-tile items)
  into kt slots of the blocks.  AV matmuls trail the exp stream by
  `lag` k-tiles.  Consecutive blocks are software-pipelined: block N+1's
  kt0 scores+exp are emitted before block N's tail-AV drain, so the
  in-order PE never sits behind the drain waiting on exp.
  Pair 1 blocks carry the output projections of earlier q-chunks; the
  last block defers half its carried outproj into the drain region to
  cover the final normalize.  Output DMAs alternate sync/scalar queues.
PSUM: scores 2x[128,1024] (4 banks) + AV accumulators 2x[65,512]
  (2 banks) + single-buffered proj/outproj [128,1024] (2 banks) = 8.
"""
import sys

sys.path.insert(0, "/opt/trn_rl_repo")

import numpy as np
import ml_dtypes

import concourse.bacc as bacc
import concourse.tile as tile
from concourse import mybir
from concourse.bass_utils import run_bass_kernel_spmd

AF = mybir.ActivationFunctionType
F32 = mybir.dt.float32
F32R = mybir.dt.float32r
I32 = mybir.dt.int32
BF = mybir.dt.bfloat16
BF_NP = ml_dtypes.bfloat16

N_CORES = 8
D = 1024          # model dim
S = 2048          # tokens per core (one batch)
E = 256           # per-core projection width (4 heads x 64)
HD = 64           # head dim
P = 128           # partitions
QC = 512          # q-chunk
SC = 1024         # projection s-chunk
DC = D // P       # 8
N_KT = S // P     # 16
N_QC = S // QC    # 4
N_SC = S // SC    # 2
EW = HD + 1       # per-head V width with ones column

SCHR_A = float(2**23 / np.log(2.0)) / 8.0          # folds the 1/8 scale
SCHR_B = float(127 * 2**23 - 0.043677448 * 2**23 + 0.5)


def build_attention_core(with_qkv_bias=False):
    scale = 1.0 / np.sqrt(np.float32(HD))

    nc = bacc.Bacc("TRN2", target_bir_lowering=False)
    xT = nc.dram_tensor("xT", [P, DC, S], BF, kind="ExternalInput")
    wq = nc.dram_tensor("wq", [P, DC, E], BF, kind="ExternalInput")
    wk = nc.dram_tensor("wk", [P, DC, E], BF, kind="ExternalInput")
    wv = nc.dram_tensor("wv", [P, DC, E], BF, kind="ExternalInput")
    wo = nc.dram_tensor("wo", [P, 2, D], BF, kind="ExternalInput")
    bq = nc.dram_tensor("bq", [P, 2], F32, kind="ExternalInput")
    bk = nc.dram_tensor("bk", [P, 2], F32, kind="ExternalInput")
    bv = nc.dram_tensor("bv", [P, 2], F32, kind="ExternalInput")
    out = nc.dram_tensor("out", [S, D], BF, kind="ExternalOutput")

    with tile.TileContext(nc) as tc:
        with (
            tc.tile_pool(name="persist", bufs=1) as persist,
            tc.tile_pool(name="attp", bufs=6) as attp,
            tc.tile_pool(name="upool", bufs=8) as upool,
            tc.tile_pool(name="u32p", bufs=1) as u32p,
            tc.tile_pool(name="urp", bufs=2) as urp,
            tc.tile_pool(name="vtrp", bufs=2) as vtrp,
            tc.tile_pool(name="small", bufs=2) as small,
            tc.tile_pool(name="outp", bufs=2) as outp,
            tc.tile_pool(name="psS", bufs=2, space="PSUM") as psS,
            tc.tile_pool(name="psP", bufs=2, space="PSUM") as psP,
            tc.tile_pool(name="psQ", bufs=1, space="PSUM") as psQ,
        ):
            # ---- input DMAs: consumption-ordered chunks on 3 queues ------
            w_sb = {}
            for nm in ("k", "v", "q"):
                w_sb[nm] = persist.tile([P, DC, E], BF, tag=f"w_{nm}",
                                        name=f"w_{nm}")
            wo_sb = persist.tile([P, 2, D], BF)
            x_sb = persist.tile([P, DC, S], BF)

            # Input DMA across three queues in first-use order.  Queue
            # discipline: ACT carries only input issues then the exp stream;
            # gpsimd only early input issues (its queue must stay clear for
            # tail broadcasts + Schraudolph copies); sync carries inputs,
            # then vflip transposes, then all output stores.
            # sync: x(sc0) o0-3 per-o (small chunks land fast), x(sc1) o0-3.
            for o in range(4):
                nc.sync.dma_start(x_sb[:, o, 0:SC], xT[:, o, 0:SC])
            for o in range(4):
                nc.sync.dma_start(x_sb[:, o, SC:2 * SC], xT[:, o, SC:2 * SC])
            # scalar: weights in use order, then x(sc1) o4-7, wo.
            nc.scalar.dma_start(w_sb["k"][:], wk[:])
            nc.scalar.dma_start(w_sb["q"][:], wq[:])
            nc.scalar.dma_start(w_sb["v"][:], wv[:])
            for o in range(4, 8):
                nc.scalar.dma_start(x_sb[:, o, SC:2 * SC], xT[:, o, SC:2 * SC])
            nc.scalar.dma_start(wo_sb[:], wo[:])
            # gpsimd SWDGE: x(sc0) o4-7 per-o, early only.
            for o in range(4, 8):
                nc.gpsimd.dma_start(x_sb[:, o, 0:SC], xT[:, o, 0:SC])

            bias_t = {}
            if with_qkv_bias:
                for nm, t in (("q", bq), ("k", bk), ("v", bv)):
                    bt = persist.tile([P, 2], F32, tag=f"b_{nm}")
                    nc.gpsimd.dma_start(bt[:], t[:])
                    bias_t[nm] = bt

            # ---- persistent activations ----------------------------------
            KT = persist.tile([P, 2, S], BF, tag="KT")   # [e, slice, s]
            QT = persist.tile([P, 2, S], BF, tag="QT")
            VT = persist.tile([P, 2, S], BF, tag="VT")
            # AV stationary: per k-chunk [V_h0|1|V_h1|1|V_h2|1|V_h3|1]
            V_sb = persist.tile([P, N_KT, 4 * EW], BF, tag="V")
            V_r = V_sb[:].rearrange("p c (h u) -> p c h u", u=EW)
            V32 = persist.tile([P, N_KT, 4 * EW], F32R, tag="V32")
            V32_r = V32[:].rearrange("p c (h u) -> p c h u", u=EW)
            V32f_r = V32[:].bitcast(F32).rearrange("p c (h u) -> p c h u", u=EW)
            for h in range(4):
                nc.gpsimd.memset(V_r[:, :, h, HD], 1.0)
                nc.gpsimd.memset(V32f_r[:, :, h, HD], 1.0)

            # ---- projection emitters (two-half items) --------------------
            dsts = {"k": KT, "q": QT, "v": VT}

            def proj_h1(nm, sl, sc, pool=None):
                s0 = sc * SC
                ps = (pool or psQ).tile([P, SC], F32,
                                        tag="S" if pool else "Q",
                                        name=f"ps_{nm}")
                for o in range(DC // 2):
                    for hh in range(SC // 512):
                        nc.tensor.matmul(
                            ps[:, hh * 512:(hh + 1) * 512],
                            w_sb[nm][:, o, sl * P:(sl + 1) * P],
                            x_sb[:, o, s0 + hh * 512:s0 + (hh + 1) * 512],
                            start=(o == 0), stop=False,
                        )
                return ps

            def proj_h2(nm, sl, sc, ps):
                s0 = sc * SC
                for o in range(DC // 2, DC):
                    for hh in range(SC // 512):
                        nc.tensor.matmul(
                            ps[:, hh * 512:(hh + 1) * 512],
                            w_sb[nm][:, o, sl * P:(sl + 1) * P],
                            x_sb[:, o, s0 + hh * 512:s0 + (hh + 1) * 512],
                            start=False, stop=(o == DC - 1),
                        )
                dst = dsts[nm][:, sl, s0:s0 + SC]
                if with_qkv_bias:
                    nc.vector.tensor_tensor(
                        dst, ps[:],
                        bias_t[nm][:, sl:sl + 1].to_broadcast((P, SC)),
                        mybir.AluOpType.add)
                else:
                    nc.vector.tensor_copy(dst, ps[:])

            def emit_proj(nm, sl, sc):
                proj_h2(nm, sl, sc, proj_h1(nm, sl, sc))

            def emit_vflip(h, sc):
                """Transpose head h's V tokens [sc*SC,(sc+1)*SC) into V_sb."""
                sl, h2 = divmod(h, 2)
                vtr = vtrp.tile([P, SC // P, HD], BF, tag="vtr")
                nc.sync.dma_start_transpose(
                    vtr[:],
                    VT[h2 * HD:(h2 + 1) * HD, sl, sc * SC:(sc + 1) * SC])
                c0 = sc * (SC // P)
                nc.vector.tensor_copy(
                    V_r[:, c0:c0 + SC // P, h, 0:HD], vtr[:])
                nc.vector.tensor_copy(
                    V32_r[:, c0:c0 + SC // P, h, 0:HD], vtr[:])

            # ---- attention -----------------------------------------------
            def emit_scores_exp(p, qc, kt, schr):
                """Scores + exp for one k-tile; returns a pend entry."""
                q0 = qc * QC
                k0 = kt * P
                st = psS.tile([P, 2 * QC], F32, tag="S", name="st")
                nc.tensor.matmul(
                    st[:, 0:QC],
                    KT[0:HD, p, k0:k0 + P], QT[0:HD, p, q0:q0 + QC],
                    tile_position=(0, 0), start=True, stop=True)
                nc.tensor.matmul(
                    st[:, QC:2 * QC],
                    KT[HD:P, p, k0:k0 + P], QT[HD:P, p, q0:q0 + QC],
                    tile_position=(64, 0), start=True, stop=True)
                if schr:
                    # Affine fp32->int32 on DVE (Pool can't read PSUM); the
                    # f32r rounding copy on the otherwise-idle GPSIMD (the
                    # BIR verifier requires a true f32r-rounding producer).
                    u32 = u32p.tile([P, 2 * QC], I32, tag="U32")
                    nc.vector.tensor_scalar(
                        u32[:], st[:], SCHR_A, SCHR_B,
                        mybir.AluOpType.mult, mybir.AluOpType.add)
                    ur = urp.tile([P, 2 * QC], F32R, tag="UR")
                    nc.gpsimd.tensor_copy(ur[:], u32[:].bitcast(F32))
                    return (kt, ur, True)
                ut = upool.tile([P, 2 * QC], BF, tag="U")
                nc.scalar.activation(ut[:], st[:], AF.Exp,
                                     scale=float(scale))
                return (kt, ut, False)

            def emit_block(p, qc, sched, lag, catchup, schr_kts,
                           head=(), next_head_fn=None):
                """Scores+exp+AV for head pair p, q-chunk qc.

                sched: {kt: [callables]} -- projection/outproj work emitted
                into that kt slot ('post' runs after the AV drain).  AV
                trails exp by `lag` k-tiles (catching up from kt=catchup).
                head: pend entries pre-emitted by the previous block.
                next_head_fn: emits the next block's kt0 just before this
                block's tail-AV drain (software pipelining).
                """
                pa = [psP.tile([EW, QC], F32, tag="P", name=f"pa{h}")
                      for h in range(2)]

                def emit_av(kt, ut, f32r):
                    vsrc = V32 if f32r else V_sb
                    for h in range(2):
                        nc.tensor.matmul(
                            pa[h][:],
                            vsrc[:, kt, (2 * p + h) * EW:(2 * p + h + 1) * EW],
                            ut[:, h * QC:(h + 1) * QC],
                            start=(kt == 0), stop=(kt == N_KT - 1))

                pend = list(head)
                for kt in range(N_KT):
                    if kt >= len(head):
                        pend.append(emit_scores_exp(p, qc, kt, kt in schr_kts))
                    hi = kt - lag
                    if catchup is not None:
                        hi += max(0, kt - catchup)
                    while pend and pend[0][0] <= hi:
                        emit_av(*pend.pop(0))
                    for fn in sched.get(kt, ()):
                        fn()
                nh = next_head_fn() if next_head_fn else ()
                for item in pend:
                    emit_av(*item)
                for fn in sched.get('post', ()):
                    fn()
                return pa, nh

            def emit_tail(p, qc, pa):
                """Normalize pair p's attention -> attnT (bf16, persists)."""
                rsb = small.tile([1, 2 * QC], F32, tag="rsb")
                for h in range(2):
                    nc.vector.tensor_copy(
                        rsb[0:1, h * QC:(h + 1) * QC], pa[h][HD:EW, :])
                rinv1 = small.tile([1, 2 * QC], F32, tag="rinv1")
                nc.vector.reciprocal_approx_fast(rinv1[:], rsb[:])
                rb = small.tile([HD, 2 * QC], F32, tag="rb")
                nc.gpsimd.partition_broadcast(rb[:], rinv1[0:1, :])
                attnT = attp.tile([P, QC], BF, tag=f"attnT_{p}_{qc}")
                for h in range(2):
                    nc.vector.tensor_tensor(
                        attnT[h * HD:(h + 1) * HD, :],
                        pa[h][0:HD, :], rb[:, h * QC:(h + 1) * QC],
                        mybir.AluOpType.mult)
                return attnT

            def out_dma(dst, src):
                # Sync only: the ACT queue carries the exp stream and the
                # gpsimd queue the tail broadcasts + Schraudolph copies.
                nc.sync.dma_start(dst, src)

            def emit_outproj_ss(qc, ss, attnT_by_p):
                q0 = qc * QC
                po = psQ.tile([P, D], F32, tag="Q", name="po")
                for p in range(2):
                    for oc in range(D // 512):
                        nc.tensor.matmul(
                            po[:, oc * 512:(oc + 1) * 512],
                            attnT_by_p[p][:, ss * P:(ss + 1) * P],
                            wo_sb[:, p, oc * 512:(oc + 1) * 512],
                            start=(p == 0), stop=(p == 1))
                osb = outp.tile([P, D], BF, tag="osb", name="osb")
                nc.vector.tensor_copy(osb[:], po[:])
                out_dma(out[q0 + ss * P:q0 + (ss + 1) * P, :], osb[:])

            def emit_outproj_final(qc, attnT_by_p):
                # Tail outproj: [128,512] units pipelined 2-deep through the
                # pa slots (free once the tail normalize has consumed them);
                # full-width bf16 rows assembled so the out DMA is contiguous.
                q0 = qc * QC
                for ss in range(QC // P):
                    osb = outp.tile([P, D], BF, tag="osb", name="osb_f")
                    for oc in range(D // 512):
                        po = psP.tile([P, 512], F32, tag="P", name="po2")
                        for p in range(2):
                            nc.tensor.matmul(
                                po[:],
                                attnT_by_p[p][:, ss * P:(ss + 1) * P],
                                wo_sb[:, p, oc * 512:(oc + 1) * 512],
                                start=(p == 0), stop=(p == 1))
                        nc.vector.tensor_copy(
                            osb[:, oc * 512:(oc + 1) * 512], po[:])
                    out_dma(out[q0 + ss * P:q0 + (ss + 1) * P, :], osb[:])

            # ---- schedule ------------------------------------------------
            # Lead-in: K00 accumulates in a psS buffer (scores haven't
            # started, the pool is idle) so Q00 in psQ isn't serialized
            # behind K00's PSUM->SBUF copy by the single psQ buffer.
            proj_h2("k", 0, 0, proj_h1("k", 0, 0, pool=psS))
            emit_proj("q", 0, 0)

            ctx = {}

            def H1(nm, sl, sc):
                return lambda: ctx.__setitem__(
                    (nm, sl, sc), proj_h1(nm, sl, sc))

            def H2(nm, sl, sc):
                return lambda: proj_h2(nm, sl, sc, ctx.pop((nm, sl, sc)))

            F = lambda h, sc: (lambda: emit_vflip(h, sc))

            attnT = {}

            def OP(qc, ss):
                return lambda: emit_outproj_ss(
                    qc, ss, (attnT[(0, qc)], attnT[(1, qc)]))

            # blocks: (p, qc, sched, lag, catchup, schr_kts)
            blocks = [
                # p0 qc0: V00 early, K01/V01 after x(sc1) lands, flips late.
                (0, 0, {2: [H1("v", 0, 0)], 3: [H2("v", 0, 0)],
                        4: [H1("k", 0, 1)], 5: [F(0, 0)],
                        6: [F(1, 0)], 7: [H2("k", 0, 1)],
                        8: [H1("v", 0, 1)], 9: [H2("v", 0, 1)],
                        11: [F(0, 1)], 12: [F(1, 1)]},
                 8, None, ()),
                (0, 1, {0: [H1("q", 0, 1)], 2: [H2("q", 0, 1)],
                        4: [H1("k", 1, 0)], 6: [H2("k", 1, 0)],
                        8: [H1("v", 1, 0)], 10: [H2("v", 1, 0)],
                        12: [F(2, 0)], 13: [F(3, 0)]},
                 5, 8, ()),
                (0, 2, {}, 5, 8, (2, 7, 12)),
                (0, 3, {0: [H1("q", 1, 0)], 2: [H2("q", 1, 0)]},
                 5, 8, (7,)),
                (1, 0, {0: [H1("k", 1, 1)], 2: [H2("k", 1, 1)],
                        4: [H1("v", 1, 1)], 6: [H2("v", 1, 1)],
                        8: [F(2, 1)], 10: [F(3, 1)]},
                 5, 8, ()),
                (1, 1, {0: [H1("q", 1, 1)], 2: [H2("q", 1, 1)],
                        4: [OP(0, 0)], 7: [OP(0, 1)],
                        10: [OP(0, 2)], 13: [OP(0, 3)]},
                 5, 8, ()),
                (1, 2, {4: [OP(1, 0)], 7: [OP(1, 1)],
                        10: [OP(1, 2)], 13: [OP(1, 3)]},
                 5, 8, (2, 9)),
                (1, 3, {4: [OP(2, 0)], 7: [OP(2, 1)],
                        10: [OP(2, 2)], 13: [OP(2, 3)]},
                 5, 8, (2, 9)),
            ]

            head = ()
            for i, (p, qc, sched, lag, cu, schr) in enumerate(blocks):
                if i + 1 < len(blocks):
                    np_, nqc, _, _, _, nschr = (
                        blocks[i + 1][0], blocks[i + 1][1], None, None, None,
                        blocks[i + 1][5])

                    def next_head_fn(np_=np_, nqc=nqc, nschr=nschr):
                        return [emit_scores_exp(np_, nqc, 0, 0 in nschr)]
                else:
                    next_head_fn = None
                pa, head = emit_block(p, qc, sched, lag, cu, schr,
                                      head=head, next_head_fn=next_head_fn)
                attnT[(p, qc)] = emit_tail(p, qc, pa)

            emit_outproj_final(3, (attnT[(0, 3)], attnT[(1, 3)]))

    nc.compile()
    return nc


_NC_CACHE = {}


def _get_nc(with_qkv_bias):
    key = with_qkv_bias
    if key not in _NC_CACHE:
        _NC_CACHE[key] = build_attention_core(with_qkv_bias)
    return _NC_CACHE[key]


def _pack_pdm(a):
    """[D, M] -> [128, D//128, M] partition-major, bf16."""
    Dd, M = a.shape
    return np.ascontiguousarray(
        a.reshape(Dd // P, P, M).transpose(1, 0, 2).astype(BF_NP))


def run_attention(x, Wq, bq, Wk, bk, Wv, bv, Wo, bo, trace=False):
    B, S_, D_ = x.shape
    assert (B, S_, D_) == (2, S, D)
    with_qkv_bias = bool(np.any(bq) or np.any(bk) or np.any(bv))
    nc = _get_nc(with_qkv_bias)
    in_maps = []
    for c in range(N_CORES):
        b, g = divmod(c, N_CORES // 2)
        sl = slice(g * E, (g + 1) * E)
        xTb = np.ascontiguousarray(x[b].T)  # [D, S]
        in_maps.append({
            "xT": _pack_pdm(xTb),
            "wq": _pack_pdm(Wq[:, sl]),
            "wk": _pack_pdm(Wk[:, sl]),
            "wv": _pack_pdm(Wv[:, sl]),
            "wo": np.ascontiguousarray(
                Wo[sl, :].reshape(2, P, D).transpose(1, 0, 2)
                .astype(BF_NP)),
            "bq": np.ascontiguousarray(
                bq[sl].reshape(2, P).T.astype(np.float32)),
            "bk": np.ascontiguousarray(
                bk[sl].reshape(2, P).T.astype(np.float32)),
            "bv": np.ascontiguousarray(
                bv[sl].reshape(2, P).T.astype(np.float32)),
        })
    res = run_bass_kernel_spmd(nc, in_maps, core_ids=list(range(N_CORES)),
                               trace=trace)
    outs = []
    for b in range(2):
        acc = np.zeros((S, D), dtype=np.float32)
        for g in range(N_CORES // 2):
            acc += np.asarray(res.results[b * 4 + g]["out"]).astype(np.float32)
        outs.append(acc + np.asarray(bo, dtype=np.float32)[None, :])
    return np.stack(outs).reshape(B, S, D), res


def kernel(x, Wq, bq, Wk, bk, Wv, bv, Wo, bo):
    out, _ = run_attention(np.asarray(x), np.asarray(Wq), np.asarray(bq),
                           np.asarray(Wk), np.asarray(bk), np.asarray(Wv),
                           np.asarray(bv), np.asarray(Wo), np.asarray(bo))
    return out
